# revision 2
# baseline (speedup 1.0000x reference)
"""Trainium2 Bass kernel for nn_CombineConcat (pairwise broadcast+concat).

reference semantics (per batch b):
  out[b, i*N + j, 0:D]   = x1[b, i, :]
  out[b, i*N + j, D:2*D] = x2[b, j, :]

Shapes (hardcoded): x1, x2 = [16, 128, 256] f32 -> out = [16, 16384, 512] f32.

Strategy: data-parallel over batch, 2 batches/core on 8 cores. Write-bound:
each core writes 64 MB. Transposed tile layout: SBUF tile T[p=i, (j, [x1|x2])]
holds, for partition i, J=16 consecutive j-slots of [x1_i | x2_j]; the output
DMA for a J-chunk writes 32KB contiguous per partition. 32KB descriptors on
2 HWDGE queues sustain ~26.4 GB/s per DMA engine = ~420 GB/s/core (vs ~300
at 2KB, ~345 at 16KB, worse at 64KB) -- the steady state is queue-drain
equilibrium: a 4MB op per queue every ~20us. Production: x2 rows are gpsimd-
partition-broadcast (6.2us/tile) from partition-0 staging chunks; x1 halves
are DVE stride-0 broadcast fills once per tile per batch. Head latency
killers: a dummy 2-channel pbcast is the first gpsimd instruction so the
~6.5us Q7 library load runs at t~3.5us instead of serializing before the
first real broadcast; the initial stage chunks load via sync/scalar (empty
at that point); first ops run j-split in halves so both queues start ~13us.
Tail: the last two ops drain j-split with halves alternated across queues.
"""

import numpy as np

_B, _N, _D = 16, 128, 256
_NCORES = 8
_BPC = _B // _NCORES  # batches per core

_NC_CACHE = {}


def _build_nc(bpc=_BPC, n=_N, d=_D, J=16, R=3, NS=3, warm=1, cool=2):
    import concourse.bacc as bacc
    import concourse.mybir as mybir
    from concourse.tile import TileContext

    assert n % J == 0
    M = n // J  # tiles per batch
    K = bpc * M  # total ops
    f32 = mybir.dt.float32
    nc = bacc.Bacc("TRN2", target_bir_lowering=False, enable_partition_id=False)
    x1 = nc.dram_tensor("x1", [bpc, n, d], f32, kind="ExternalInput")
    x2 = nc.dram_tensor("x2", [bpc, n, d], f32, kind="ExternalInput")
    out = nc.dram_tensor("out", [bpc, n * n, 2 * d], f32, kind="ExternalOutput")
    W = 2 * d  # output row width (elements)

    with TileContext(nc) as tc:
        with (
            tc.tile_pool(name="io", bufs=1) as iop,
            tc.tile_pool(name="ring", bufs=1) as rp,
        ):
            oqs = [nc.sync, nc.scalar]
            # Dummy first gpsimd op: pulls the Q7 extended-inst library load
            # (~6.5us) to the very start of the kernel where it overlaps the
            # input staging DMAs.
            dum = iop.tile([2, 1], f32, name="dum", tag="dum")
            nc.gpsimd.partition_broadcast(dum[0:2, :], dum[0:1, :], opt=False)

            stages = [
                iop.tile([1, J * d], f32, name=f"x2st_{s}", tag=f"x2st_{s}")
                for s in range(NS)
            ]
            x2f = [x2[b].rearrange("n d -> (n d)") for b in range(bpc)]

            def stage_load(k, half=None, eng=None):
                b, m = divmod(k, M)
                lo = m * J * d
                hi = (m + 1) * J * d
                mid = (lo + hi) // 2
                off = 0
                if half == 0:
                    hi = mid
                elif half == 1:
                    lo = mid
                    off = J * d // 2
                (eng or nc.gpsimd).dma_start(
                    out=stages[k % NS][0:1, off : off + (hi - lo)],
                    in_=x2f[b][lo:hi],
                )

            # Initial stage chunks on the (still idle) output queues.
            for k in range(min(NS, K)):
                if k < warm:
                    stage_load(k, half=0, eng=oqs[k % 2])
                    stage_load(k, half=1, eng=oqs[(k + 1) % 2])
                else:
                    stage_load(k, eng=oqs[k % 2])
            x1sb = []
            for b in range(bpc):
                t = iop.tile([n, d], f32, name=f"x1sb_{b}", tag=f"x1sb_{b}")
                oqs[b % 2].dma_start(out=t[:], in_=x1[b])
                x1sb.append(t)
            tiles = [rp.tile([n, J * W], f32, name=f"T{r}", tag=f"T{r}") for r in range(R)]
            views = [t[:].rearrange("p (j two c) -> p j two c", j=J, c=d) for t in tiles]

            k = 0
            for b in range(bpc):
                obv = out[b].rearrange("(i j) w -> i j w", i=n)
                for m in range(M):
                    s = k % NS
                    r = k % R
                    if k < R or (b > 0 and m < R):
                        nc.vector.tensor_copy(
                            out=views[r][:, :, 0, :],
                            in_=x1sb[b][:].unsqueeze(1).broadcast_to([n, J, d]),
                        )
                    sv = stages[s][0:1, :].rearrange("p (j c) -> p j c", j=J)
                    head = k < warm
                    tail = k >= K - cool
                    if head or tail:
                        # j-split: produce + write in two halves. Head ops put
                        # both halves on their own queue (starts both queues
                        # early); tail ops alternate halves across queues so
                        # the final bytes drain balanced.
                        hj = J // 2
                        for h in range(2):
                            nc.gpsimd.partition_broadcast(
                                views[r][:, h * hj : (h + 1) * hj, 1, :],
                                sv[:, h * hj : (h + 1) * hj, :],
                                opt=False,
                            )
                            q = (k + h) % 2
                            oqs[q].dma_start(
                                out=obv[:, m * J + h * hj : m * J + (h + 1) * hj, :],
                                in_=tiles[r][:, h * hj * W : (h + 1) * hj * W],
                            )
                    else:
                        nc.gpsimd.partition_broadcast(
                            views[r][:, :, 1, :], sv, opt=False
                        )
                        oqs[(k + 1) % 2].dma_start(
                            out=obv[:, m * J : (m + 1) * J, :], in_=tiles[r][:]
                        )
                    if k + 2 < K and k + 2 >= min(NS, K):
                        stage_load(k + 2)
                    k += 1
    nc.finalize()
    return nc


def _get_nc():
    if "nc" not in _NC_CACHE:
        _NC_CACHE["nc"] = _build_nc()
    return _NC_CACHE["nc"]


def _run(x1, x2, trace=False):
    """Run the kernel on 8 cores; returns (output, BassKernelResults)."""
    from concourse.bass_utils import run_bass_kernel_spmd

    nc = _get_nc()
    x1 = np.ascontiguousarray(np.asarray(x1, dtype=np.float32))
    x2 = np.ascontiguousarray(np.asarray(x2, dtype=np.float32))
    in_maps = [
        {
            "x1": x1[c * _BPC : (c + 1) * _BPC],
            "x2": x2[c * _BPC : (c + 1) * _BPC],
        }
        for c in range(_NCORES)
    ]
    res = run_bass_kernel_spmd(
        nc, in_maps, core_ids=list(range(_NCORES)), trace=trace
    )
    out = np.concatenate([r["out"] for r in res.results], axis=0)
    return out, res


def kernel(x1, x2):
    out, _ = _run(x1, x2, trace=False)
    return out
